# revision 43
# baseline (speedup 1.0000x reference)
"""Trainium2 Bass kernel for nn_AttentionV2 (dense transformer attention block).

Reference computation (B=4, C=256, H=W=48, heads=8, d=32, N=2304):
  qk   = conv1x1(x, w_qk) -> q,k per head [d, N]
  v4   = conv1x1(x, w_v)
  pe   = dwconv3x3(v4, w_pe)            (depthwise, SAME)
  S    = q^T k * d^-0.5 ; P = softmax_j(S)
  out  = v @ P^T  (per head)
  y    = conv1x1(out + pe, w_proj)

Sharding: 8 cores = 4 batches x 2 spatial halves (rows 0-23 / 24-47).
Each core computes full k,v for its batch -> zero collectives. Per-core x is
ROLLED by y0 rows so the SPMD program always works on "rows 0..23".

v2: full fp8e4 attention via dual-fp8 (DoubleRow) matmuls:
 - ST: full-K bf16 matmuls on zero-padded per-head k (k_pad): the PE power
   governor (HAM) clamps the clock to 4/8 duty unless it sees sustained
   full-row matmul activity, so bf16 full-K ST is effectively free vs fp8
   (same stream cost once the clock is halved otherwise).
 - exp: split between ACT (native Exp -> fp8 out) and DVE (Schraudolph:
   round(A*S + B) written as int8 = the fp8e4 bit pattern; HW rounds).
 - AV: lhsT = vT [128, 2, 128] per (head, j-pair): cols 0:32 = v, col 32 =
   1.0 (softmax denominator via the accumulated row 32), cols 33:127 = 1.0
   pad (free size must be exactly 128 for dual-fp8; pad rows of the psum
   output are never read). E is the fp8 exp output, streamed 2 cols/cycle.
 - v bias bv folds into the norm step (sum_j P = 1), as in the reference
   repack; pe/taps run in bf16 on DVE for the 2x 16-bit mode.
"""

import numpy as np
from ml_dtypes import bfloat16

C = 256
HW = 48
N = HW * HW          # 2304
NH = N // 2          # 1152 per-core i-pixels
NHEADS = 8
D = 32
SCALE = D ** -0.5
NJ = N // 128        # 18 j-chunks
NU = NJ // 2         # 9 j-chunk pairs
ISUBS = [(0, 384), (384, 384), (768, 384)]
A_SCH = 8 * np.log2(np.e) * SCALE   # Schraudolph slope (HW rounds f32->int8)
B_SCH = 55.85
WPACK = 160
WPH = 1024

_CACHE = {}


def _build_bass():
    import concourse.bass as bass
    import concourse.bacc as bacc
    import concourse.mybir as mybir
    from concourse import tile

    f32 = mybir.dt.float32
    bf16 = mybir.dt.bfloat16
    f8 = mybir.dt.float8e4
    i8 = mybir.dt.int8
    u16 = mybir.dt.uint16
    AF = mybir.ActivationFunctionType
    OP = mybir.AluOpType
    DR = mybir.MatmulPerfMode.DoubleRow

    nc = bacc.Bacc()

    x_d = nc.dram_tensor("x", [C, N], bf16, kind="ExternalInput")
    wpack_d = nc.dram_tensor("wpack", [C, WPACK], f32, kind="ExternalInput")
    wpackh_d = nc.dram_tensor("wpackh", [C, WPH], bf16, kind="ExternalInput")
    out_d = nc.dram_tensor("out", [C, NH], f32, kind="ExternalOutput")
    import os as _os
    DBG = _os.environ.get("KDBG", "0") == "1"
    ALL_ACT = _os.environ.get("ALL_ACT", "0") == "1"
    if DBG:
        dq_d = nc.dram_tensor("dbg_q", [C, 3 * 2 * 384], mybir.dt.uint8, kind="ExternalOutput")
        dk_d = nc.dram_tensor("dbg_k", [C, NJ * 2 * 128], mybir.dt.uint8, kind="ExternalOutput")
        dvt_d = nc.dram_tensor("dbg_vt", [128, NU * NHEADS * 2 * 128], mybir.dt.uint8, kind="ExternalOutput")
        dou_d = nc.dram_tensor("dbg_outU", [C, NH], f32, kind="ExternalOutput")
        dl_d = nc.dram_tensor("dbg_l", [C, NH], f32, kind="ExternalOutput")
        dpin_d = nc.dram_tensor("dbg_pin", [C, NH], f32, kind="ExternalOutput")
        dav_d = nc.dram_tensor("dbg_av", [C, 384], f32, kind="ExternalOutput")
        det_d = nc.dram_tensor("dbg_et", [C, 2 * 384], mybir.dt.uint8, kind="ExternalOutput")

    with tile.TileContext(nc) as tc:
        with (
            tc.tile_pool(name="wts", bufs=1) as wp,
            tc.tile_pool(name="per", bufs=1) as pp,
        ):
            # ---- persistent weight/bias tiles
            wsb = [wp.tile([128, WPACK], f32, tag=f"wsb{c}", name=f"wsb{c}") for c in range(2)]
            wph = [wp.tile([128, WPH], bf16, tag=f"wph{c}", name=f"wph{c}") for c in range(2)]
            for c in range(2):
                nc.sync.dma_start(out=wsb[c][:, :], in_=wpack_d[128 * c:128 * (c + 1), :])
            # weight DMA order: k block first (gates the first conv), then
            # q, wv; wproj is dispatched after the prefix (sync-queue
            # dispatch is ~650ns per DMA, so order = priority)
            for c in range(2):
                nc.sync.dma_start(out=wph[c][:, 256:512],
                                  in_=wpackh_d[128 * c:128 * (c + 1), 256:512])
            # col layout of wph: 0:256 wqT (by-head), 256:512 wkT,
            # 512:768 wvT, 768:1024 wprojT
            wpe = [wsb[c][:, 0:9] for c in range(2)]
            bv = [wsb[c][:, 9:10] for c in range(2)]
            bvpe = [wsb[c][:, 10:11] for c in range(2)]
            bproj = [wsb[c][:, 11:12] for c in range(2)]
            bvht = [wsb[c][:, 12:13] for c in range(2)]
            halo = [wsb[c][:, 13:15] for c in range(2)]
            bq = [wsb[c][:, 15:16] for c in range(2)]
            bk = [wsb[c][:, 144:145] for c in range(2)]
            bcastW = wsb[0][:, 16:144]

            # ---- persistent activations
            q_sb = [pp.tile([128, NH], bf16, tag=f"q{c}", name=f"q{c}") for c in range(2)]
            k_sb = [pp.tile([128, N], bf16, tag=f"k{c}", name=f"k{c}") for c in range(2)]
            k_pad = [pp.tile([128, N], bf16, tag=f"kp{h}", name=f"kp{h}")
                     for h in range(NHEADS)]
            vT = pp.tile([128, NU, NHEADS, 2, 128], f8, tag="vT", name="vT")
            v4 = [pp.tile([128, 26, HW], bf16, tag=f"v4{c}", name=f"v4{c}") for c in range(2)]
            htop = [pp.tile([128, 1, HW], bf16, tag=f"htop{c}", name=f"htop{c}") for c in range(2)]
            hbot = [pp.tile([128, 1, HW], bf16, tag=f"hbot{c}", name=f"hbot{c}") for c in range(2)]
            pe = [pp.tile([128, 24, HW], bf16, tag=f"pe{c}", name=f"pe{c}") for c in range(2)]
            outU = [pp.tile([128, NH], f32, tag=f"outU{c}", name=f"outU{c}") for c in range(2)]
            # per-head denominators live at 32-aligned rows (32*g) -- DVE
            # partition offsets must be quad-aligned, and DVE cost depends
            # only on the free size, so full-height tiles are free
            lsc = [pp.tile([128, NH], f32, tag=f"l{g}", name=f"l{g}") for g in range(2)]
            rlsc = [pp.tile([128, NH], f32, tag=f"rl{g}", name=f"rl{g}") for g in range(2)]
            rscr = pp.tile([128, NH], f32, tag="rscr", name="rscr")
            rlb_sb = [pp.tile([128, NH], f32, tag=f"rlb{g}", name=f"rlb{g}") for g in range(2)]
            proj_in = [pp.tile([128, NH], bf16, tag=f"pin{c}", name=f"pin{c}") for c in range(2)]
            y_sb = [pp.tile([128, NH], f32, tag=f"y{c}", name=f"y{c}") for c in range(2)]

            # scratch for the PE warm-up burst (memset first so the burst
            # matmuls unblock immediately; on GpSimd so DVE stays free)
            wtmp = pp.tile([128, 512], bf16, tag="wtmp", name="wtmp")
            nc.gpsimd.memset(wtmp[:, :], 1.0)
            # dependency-free dummy exp pulls the ACT_TABLE_LOAD off the
            # first real exp's critical path
            expw = pp.tile([128, 16], f32, tag="expw", name="expw")
            nc.scalar.activation(expw[:, :], wtmp[:, 0:16], AF.Exp, scale=1.0)
            # vT = 1.0 everywhere: col 32 becomes the denominator ones-row,
            # cols 33:127 are pad (their psum rows are unread), v cols get
            # overwritten by the vt copies. u16 view for the DVE 2x mode.
            nc.vector.memset(vT[:, 0, :, :, :].bitcast(u16), 0x3838)
            # k_pad zero fill: rows outside each head's 32 contract against
            # other heads' q rows and must be 0. Pool engine is idle; heads
            # 0/1 first (they gate the very first ST).
            for h in range(NHEADS):
                nc.gpsimd.memset(k_pad[h][:, :], 0.0)
            # never-written lsc rows must be finite: the fast-path broadcast
            # matmul contracts all 128 partitions (0 * NaN = NaN)
            for g in range(2):
                nc.gpsimd.memset(lsc[g][:, :], 1.0)

            # ================= stage 1: minimal prefix =================
            xpool = tc.tile_pool(name="xp", bufs=1)
            xp = xpool.__enter__()
            x_sb = [xp.tile([128, N], bf16, tag=f"x{c}", name=f"x{c}") for c in range(2)]
            for c in range(2):
                nc.sync.dma_start(out=x_sb[c][:, 0:384],
                                  in_=x_d[128 * c:128 * (c + 1), 0:384])
            for (c0, w) in [(0, 256), (512, 256)]:
                for c in range(2):
                    nc.sync.dma_start(out=wph[c][:, c0:c0 + w],
                                      in_=wpackh_d[128 * c:128 * (c + 1), c0:c0 + w])

            def spt(pool):
                return pool.tile([128, 2, 512], f32, tag="st", name="st")[:, 0, :]

            def q_chunk(oc, win, pool):
                pt = spt(pool)
                i0, w = ISUBS[win]
                for c in range(2):
                    nc.tensor.matmul(
                        pt[:, :w], wph[c][:, 128 * oc:128 * (oc + 1)],
                        x_sb[c][:, i0:i0 + w],
                        start=(c == 0), stop=(c == 1),
                    )
                nc.scalar.activation(q_sb[oc][:, i0:i0 + w], pt[:, :w],
                                     AF.Identity, bias=bq[oc][:, :], scale=1.0)

            def k_chunk(oc, ch, pool):
                pt = spt(pool)
                c0 = 384 * ch
                for c in range(2):
                    nc.tensor.matmul(
                        pt[:, :384], wph[c][:, 256 + 128 * oc:256 + 128 * (oc + 1)],
                        x_sb[c][:, c0:c0 + 384],
                        start=(c == 0), stop=(c == 1),
                    )
                nc.scalar.activation(k_sb[oc][:, c0:c0 + 384], pt[:, :384],
                                     AF.Identity, bias=bk[oc][:, :], scale=1.0)

            def kpad_dma(h, ch=None):
                # copy head h's 32 k rows into its zero-padded full-K tile
                r = 32 * (h % 4)
                sl = slice(0, N) if ch is None else slice(384 * ch, 384 * (ch + 1))
                nc.sync.dma_start(out=k_pad[h][r:r + 32, sl],
                                  in_=k_sb[h // 4][r:r + 32, sl])

            def vt_chunk(j, pool):
                pt = spt(pool)
                for c in range(2):
                    nc.tensor.matmul(
                        pt[:, :256],
                        x_sb[c][:, 128 * j:128 * (j + 1)],
                        wph[c][:, 512:768],
                        start=(c == 0), stop=(c == 1),
                    )
                nc.vector.tensor_copy(
                    vT[:, j // 2, :, j % 2, 0:32],
                    pt[:, :256].rearrange("p (h d) -> p h d", d=32),
                )

            def v4_chunk(oc, c0, w, pool):
                pt = spt(pool)
                for c in range(2):
                    nc.tensor.matmul(
                        pt[:, :w],
                        wph[c][:, 512 + 128 * oc:512 + 128 * (oc + 1)],
                        x_sb[c][:, c0:c0 + w],
                        start=(c == 0), stop=(c == 1),
                    )
                v4f = v4[oc][:, :, :].rearrange("p a b -> p (a b)")
                nc.vector.tensor_scalar(v4f[:, c0:c0 + w], pt[:, :w], bv[oc][:, :], None, OP.add)

            def v4_halo(oc, pool):
                pt = spt(pool)
                for c in range(2):
                    nc.tensor.matmul(
                        pt[:, :48],
                        wph[c][:, 512 + 128 * oc:512 + 128 * (oc + 1)],
                        x_sb[c][:, 47 * 48:48 * 48],
                        start=(c == 0), stop=(c == 1),
                    )
                nc.scalar.activation(htop[oc][:, 0, :], pt[:, :48], AF.Identity,
                                     bias=bvht[oc][:, :], scale=halo[oc][:, 0:1])
                nc.scalar.activation(hbot[oc][:, 0, :], v4[oc][:, 24, :], AF.Copy,
                                     scale=halo[oc][:, 1:2])

            TAPS = [
                (-1, -1, (1, 24), (0, 23), (1, 48), (0, 47)),
                (-1, 0, (1, 24), (0, 23), (0, 48), (0, 48)),
                (-1, 1, (1, 24), (0, 23), (0, 47), (1, 48)),
                (0, -1, (0, 24), (0, 24), (1, 48), (0, 47)),
                (0, 1, (0, 24), (0, 24), (0, 47), (1, 48)),
                (1, -1, (0, 23), (1, 24), (1, 48), (0, 47)),
                (1, 0, (0, 23), (1, 24), (0, 48), (0, 48)),
                (1, 1, (0, 23), (1, 24), (0, 47), (1, 48)),
            ]

            def pe_center(oc):
                nc.vector.tensor_scalar(pe[oc][:, :, :], v4[oc][:, 0:24, :],
                                        wpe[oc][:, 4:5], None, OP.mult)

            def pe_tap(oc, ti):
                (dy, dx, oy, iy, ox, ix) = TAPS[ti]
                acc = pe[oc]
                src = v4[oc]
                wap = wpe[oc][:, 3 * (dy + 1) + (dx + 1):3 * (dy + 1) + (dx + 1) + 1]
                nc.vector.scalar_tensor_tensor(
                    acc[:, oy[0]:oy[1], ox[0]:ox[1]],
                    src[:, iy[0]:iy[1], ix[0]:ix[1]],
                    wap,
                    acc[:, oy[0]:oy[1], ox[0]:ox[1]],
                    OP.mult, OP.add,
                )

            def pe_edges(oc):
                acc = pe[oc]
                for (dx, ox, ix) in [(-1, (1, 48), (0, 47)), (0, (0, 48), (0, 48)), (1, (0, 47), (1, 48))]:
                    wap = wpe[oc][:, (dx + 1):(dx + 2)]
                    nc.vector.scalar_tensor_tensor(
                        acc[:, 0:1, ox[0]:ox[1]], htop[oc][:, :, ix[0]:ix[1]],
                        wap, acc[:, 0:1, ox[0]:ox[1]], OP.mult, OP.add,
                    )
                    wap = wpe[oc][:, 6 + (dx + 1):6 + (dx + 2)]
                    nc.vector.scalar_tensor_tensor(
                        acc[:, 23:24, ox[0]:ox[1]], hbot[oc][:, :, ix[0]:ix[1]],
                        wap, acc[:, 23:24, ox[0]:ox[1]], OP.mult, OP.add,
                    )

            with tc.tile_pool(name="ps1", bufs=2, space="PSUM") as ps1:
                # PE warm-up burst: back-to-back matmuls independent of input
                # DMA, fires the HAM un-throttle before the real convs
                for _ in range(10):
                    pw = spt(ps1)
                    nc.tensor.matmul(pw[:, :384], wtmp[:, 0:128],
                                     wtmp[:, 0:384], start=True, stop=True)
                k_chunk(0, 0, ps1)
                kpad_dma(0, 0)
                kpad_dma(1, 0)
                q_chunk(0, 0, ps1)
                vt_chunk(0, ps1)
                vt_chunk(1, ps1)
                # bulk x + wproj dispatch AFTER the prefix's critical DMAs
                # (the sync queue issues ~650ns per dma_start in order, so
                # the kpad copies must not sit behind these)
                for s in range(1, 6):
                    for c in range(2):
                        nc.sync.dma_start(out=x_sb[c][:, 384 * s:384 * (s + 1)],
                                          in_=x_d[128 * c:128 * (c + 1), 384 * s:384 * (s + 1)])
                for c in range(2):
                    nc.sync.dma_start(out=wph[c][:, 768:1024],
                                      in_=wpackh_d[128 * c:128 * (c + 1), 768:1024])

            with tc.tile_pool(name="drp", bufs=1, space="DRAM") as drp:
                rld = drp.tile([NHEADS, NH], f32, tag="rld", name="rld")

            # ================= stage 2: attention + fillers ============
            with (
                tc.tile_pool(name="ep", bufs=5) as ep,
                tc.tile_pool(name="stp", bufs=3, space="PSUM") as stp,
                tc.tile_pool(name="ava", bufs=1, space="PSUM") as ava,
            ):
                ps3 = stp
                def emit_norm(oc, i0, icw, fast=False):
                    # denominators -> reciprocals -> broadcast -> normalize,
                    # + bv + positional conv. Steady state broadcasts via a
                    # DRAM roundtrip; the final window uses one fp32 PE
                    # matmul with the block pattern instead.
                    nc.vector.reciprocal_approx_accurate(rlsc[oc][:, i0:i0 + icw],
                                                         lsc[oc][:, i0:i0 + icw],
                                                         rscr[:, i0:i0 + icw])
                    if fast:
                        rlb_ps = spt(stp)
                        nc.tensor.matmul(
                            rlb_ps[:, :icw],
                            bcastW[:, :],
                            rlsc[oc][:, i0:i0 + icw],
                            start=True, stop=True,
                        )
                        rlb = rlb_ps[:, :icw]
                    else:
                        for g in range(4):
                            h = 4 * oc + g
                            nc.sync.dma_start(out=rld[h:h + 1, i0:i0 + icw],
                                              in_=rlsc[oc][32 * g:32 * g + 1, i0:i0 + icw])
                            nc.sync.dma_start(
                                out=rlb_sb[oc][32 * g:32 * (g + 1), i0:i0 + icw],
                                in_=rld[h:h + 1, i0:i0 + icw].partition_broadcast(32),
                            )
                        rlb = rlb_sb[oc][:, i0:i0 + icw]
                    pef = pe[oc][:, :, :].rearrange("p a b -> p (a b)")
                    nc.vector.tensor_tensor(
                        proj_in[oc][:, i0:i0 + icw], outU[oc][:, i0:i0 + icw],
                        rlb, OP.mult,
                    )
                    nc.vector.scalar_tensor_tensor(
                        proj_in[oc][:, i0:i0 + icw], proj_in[oc][:, i0:i0 + icw],
                        bvpe[oc][:, :], pef[:, i0:i0 + icw], OP.add, OP.add,
                    )

                def emit_proj(cin, i0, icw):
                    # proj partial for input-channel half `cin`: accumulate
                    # into y_sb (cin==0 adds the bias, cin==1 adds psum) and
                    # DMA out when complete
                    for oc in range(2):
                        pt = spt(stp)
                        nc.tensor.matmul(
                            pt[:, :icw],
                            wph[cin][:, 768 + 128 * oc:768 + 128 * (oc + 1)],
                            proj_in[cin][:, i0:i0 + icw],
                            start=True, stop=True,
                        )
                        if cin == 0:
                            nc.vector.tensor_scalar(y_sb[oc][:, i0:i0 + icw], pt[:, :icw],
                                                    bproj[oc][:, :], None, OP.add)
                        else:
                            nc.vector.tensor_tensor(y_sb[oc][:, i0:i0 + icw],
                                                    y_sb[oc][:, i0:i0 + icw],
                                                    pt[:, :icw], OP.add)
                            nc.sync.dma_start(out=out_d[128 * oc:128 * (oc + 1), i0:i0 + icw],
                                              in_=y_sb[oc][:, i0:i0 + icw])

                # filler lists per (grp, window). grp0-w0 is handled inline
                # (ordering constraints); these lists fill the rest.
                FILL = {}
                FILL[(0, 1)] = ([lambda: q_chunk(0, 2, ps3)]
                                + [(lambda c0=c0, w=w: v4_chunk(0, c0, w, ps3))
                                   for (c0, w) in [(0, 384), (384, 384), (768, 384), (1152, 48)]]
                                + [lambda: v4_halo(0, ps3)])
                FILL[(0, 2)] = ([(lambda ch=ch: k_chunk(1, ch, ps3))
                                 for ch in range(3)]
                                + [lambda: pe_center(0)]
                                + [lambda ti=ti: pe_tap(0, ti) for ti in range(4)])
                FILL[(1, 0)] = ([(lambda ch=ch: k_chunk(1, ch, ps3))
                                 for ch in range(3, 6)]
                                + [lambda: q_chunk(1, 0, ps3)]
                                + [lambda ti=ti: pe_tap(0, ti) for ti in range(4, 8)]
                                + [lambda: pe_edges(0)])
                FILL[(1, 1)] = ([lambda: q_chunk(1, 1, ps3),
                                 lambda: kpad_dma(4), lambda: kpad_dma(5)]
                                + [(lambda c0=c0, w=w: v4_chunk(1, c0, w, ps3))
                                   for (c0, w) in [(0, 384), (384, 384), (768, 384), (1152, 48)]]
                                + [lambda: v4_halo(1, ps3)])
                FILL[(1, 2)] = ([lambda: q_chunk(1, 2, ps3),
                                 lambda: kpad_dma(6), lambda: kpad_dma(7)]
                                + [lambda: pe_center(1)]
                                + [lambda ti=ti: pe_tap(1, ti) for ti in range(4)])
                # oc0 norms + proj c0 partials spread over grp2/grp3
                FILL[(2, 0)] = ([lambda ti=ti: pe_tap(1, ti) for ti in range(4, 8)]
                                + [lambda: pe_edges(1)]
                                + [lambda: emit_norm(0, 0, 384)])
                FILL[(2, 1)] = [lambda: emit_norm(0, 384, 384), lambda: emit_proj(0, 0, 384)]
                FILL[(2, 2)] = [lambda: emit_norm(0, 768, 384), lambda: emit_proj(0, 384, 384)]
                FILL[(3, 0)] = [lambda: emit_proj(0, 768, 384)]
                FILL[(3, 1)] = [lambda: emit_norm(1, 0, 384)]
                FILL[(3, 2)] = [lambda: emit_norm(1, 384, 384),
                                lambda: emit_proj(1, 0, 384)]

                # grp0-w0 fillers with ordering constraints: vt chunk 2u must
                # land before AV(pair u), k chunk c before the pair reading
                # j=3c, q-win1 before grp0-w1.
                def vtm(u):
                    # deferred vT ones/pad memset (u16 view for DVE 2x);
                    # must precede vt chunks 2u/2u+1 and AV(u)
                    nc.vector.memset(vT[:, u, :, :, :].bitcast(u16), 0x3838)

                def kc0(ch):
                    k_chunk(0, ch, ps3)
                    kpad_dma(0, ch)
                    kpad_dma(1, ch)

                w0fill = [
                    [lambda: vtm(1), lambda: vtm(2), lambda: kc0(1)],                 # after p0
                    [lambda: vtm(3), lambda: vtm(4), lambda: vtm(5), lambda: vtm(6),
                     lambda: vtm(7), lambda: vtm(8),
                     lambda: vt_chunk(2, ps3), lambda: vt_chunk(3, ps3)],             # p1
                    [lambda: kc0(2), lambda: vt_chunk(4, ps3), lambda: vt_chunk(5, ps3)],
                    [lambda: kc0(3), lambda: vt_chunk(6, ps3), lambda: vt_chunk(7, ps3)],
                    [lambda: vt_chunk(8, ps3), lambda: vt_chunk(9, ps3)],             # p4
                    [lambda: kc0(4), lambda: vt_chunk(10, ps3), lambda: vt_chunk(11, ps3)],
                    [lambda: kc0(5), lambda: vt_chunk(12, ps3), lambda: vt_chunk(13, ps3)],
                    [lambda: vt_chunk(14, ps3), lambda: vt_chunk(15, ps3),
                     lambda: kpad_dma(2)],                                            # p7
                    [lambda: vt_chunk(16, ps3), lambda: vt_chunk(17, ps3),
                     lambda: kpad_dma(3)],                                            # p8
                    [lambda: q_chunk(0, 1, ps3)],                                     # p9
                ]

                for grp in range(4):
                    heads = [2 * grp, 2 * grp + 1]
                    oc = heads[0] // 4
                    gs = [h % 4 for h in heads]
                    for ici, (i0, icw) in enumerate(ISUBS):
                        if grp == 0 and ici == 0:
                            fillers = None
                        else:
                            fillers = list(FILL.get((grp, ici), []))
                        fi = 0
                        avl = {h: ava.tile([128, 512], f32, tag=f"avl{h % 2}", name=f"avl{h % 2}")
                               for h in heads}
                        ets = {}
                        pairs = []
                        for u in range(NU):
                            for h in heads:
                                pairs.append((h, u))

                        def emit_st(p, t):
                            (h, u) = p
                            # 512-wide halves: each matmul output must START
                            # at a PSUM bank boundary (start=True zeroing is
                            # bank-granular from the start address)
                            st = stp.tile([128, 2, 512], f32, tag="st", name="st")
                            for half in range(2):
                                jc = 2 * u + half
                                nc.tensor.matmul(
                                    st[:, half, :icw],
                                    k_pad[h][:, 128 * jc:128 * (jc + 1)],
                                    q_sb[oc][:, i0:i0 + icw],
                                    start=True, stop=True,
                                )
                            et = ep.tile([128, 2, 384], f8, tag="E", name="E")
                            # exp split: ~2/3 on ACT (native exp), rest on DVE
                            # via the Schraudolph bit trick
                            if (t % 3) == 2 and not ALL_ACT:
                                nc.vector.tensor_scalar(
                                    et[:, :, :icw].bitcast(i8), st[:, :, :icw],
                                    float(A_SCH), float(B_SCH), OP.mult, OP.add)
                            else:
                                nc.scalar.activation(et[:, :, :icw], st[:, :, :icw],
                                                     AF.Exp, scale=SCALE)
                            if DBG and grp == 0 and ici == 0 and p in ((0, 0), (0, 4)):
                                nc.sync.dma_start(
                                    out=det_d[128 * (p[1] // 4):128 * (p[1] // 4 + 1), :],
                                    in_=et[:, :, :].bitcast(mybir.dt.uint8).rearrange("p a b -> p (a b)"))
                            ets[p] = et

                        def emit_av(p):
                            (h, u) = p
                            et = ets.pop(p)
                            nc.tensor.matmul(
                                avl[h][:, :icw],
                                vT[:, u, h, :, :],
                                et[:, :, :icw],
                                start=(u == 0), stop=(u == NU - 1),
                                perf_mode=DR,
                            )

                        for t, p in enumerate(pairs):
                            emit_st(p, t)
                            if t >= 1:
                                emit_av(pairs[t - 1])
                            if grp == 0 and ici == 0:
                                if t < len(w0fill):
                                    for f in w0fill[t]:
                                        f()
                            else:
                                for _ in range(2):
                                    if fi < len(fillers):
                                        fillers[fi]()
                                        fi += 1
                        if fillers:
                            while fi < len(fillers):
                                fillers[fi]()
                                fi += 1
                        emit_av(pairs[-1])

                        if DBG and grp == 0 and ici == 0:
                            for hi, h in enumerate(heads):
                                dava = pp.tile([128, 384], f32, tag=f"dava{hi}", name=f"dava{hi}")
                                nc.vector.tensor_copy(dava[:, :], avl[h][:, :])
                                nc.sync.dma_start(out=dav_d[128 * hi:128 * (hi + 1), :],
                                                  in_=dava[:, :])
                        # drain: rows 0:32 -> outU, row 32 (denominator) ->
                        # l_g; both partition-shifted DVE copies (DMA cannot
                        # read PSUM)
                        for h in heads:
                            g = h % 4
                            nc.scalar.copy(outU[oc][32 * g:32 * (g + 1), i0:i0 + icw],
                                           avl[h][0:32, :icw])
                            nc.vector.tensor_copy(lsc[oc][32 * g:32 * g + 1, i0:i0 + icw],
                                                  avl[h][32:33, :icw])
                if DBG:
                    dbs = pp.tile([128, NH], f32, tag="dbs", name="dbs")
                    for oc in range(2):
                        nc.sync.dma_start(out=dou_d[128 * oc:128 * (oc + 1), :], in_=outU[oc][:, :])
                        nc.sync.dma_start(out=dl_d[128 * oc:128 * (oc + 1), :], in_=lsc[oc][:, :])
                        nc.vector.tensor_copy(dbs[:, :], proj_in[oc][:, :])
                        nc.sync.dma_start(out=dpin_d[128 * oc:128 * (oc + 1), :], in_=dbs[:, :])
                    nc.sync.dma_start(out=dvt_d[:, :],
                                      in_=vT[:, :, :, :, :].bitcast(mybir.dt.uint8).rearrange("p a b c d -> p (a b c d)"))
                # tail: final window's norm + last proj partials
                emit_norm(1, 768, 384, fast=True)
                emit_proj(1, 384, 384)
                emit_proj(1, 768, 384)
            xpool.__exit__(None, None, None)

    nc.finalize()
    return nc


def _prep_inputs(x, w_qk, b_qk, w_v, b_v, w_pe, b_pe, w_proj, b_proj):
    f = np.float32
    wq2 = w_qk[:, :, 0, 0].reshape(NHEADS, 2 * D, C).astype(f)
    bq2 = b_qk.reshape(NHEADS, 2 * D).astype(f)

    wpackh = np.zeros((C, WPH), dtype=f)
    # q/k conv out channel p = head (p//32) of its oc, d = p%32
    wq = wq2[:, :D].reshape(C, C)     # [head-major out ch, C]
    wk = wq2[:, D:].reshape(C, C)
    wpackh[:, 0:256] = wq.T
    wpackh[:, 256:512] = wk.T
    wpackh[:, 512:768] = w_v[:, :, 0, 0].T
    wpackh[:, 768:1024] = w_proj[:, :, 0, 0].T
    wpackh = np.ascontiguousarray(wpackh.astype(bfloat16))

    base = np.zeros((C, WPACK), dtype=f)
    base[:, 0:9] = w_pe[:, 0].reshape(C, 9)
    base[:, 9] = b_v
    base[:, 10] = b_v + b_pe
    base[:, 11] = b_proj
    base[:, 15] = bq2[:, :D].reshape(C)
    base[:, 144] = bq2[:, D:].reshape(C)
    for p in range(128):
        for cc in range(128):
            if p == 32 * (cc // 32):
                base[p, 16 + cc] = 1.0

    in_maps = []
    for core in range(8):
        b, half = core // 2, core % 2
        y0 = 24 * half
        xb = x[b].reshape(C, HW, HW).astype(f)
        xr = np.concatenate([xb[:, y0:, :], xb[:, :y0, :]], axis=1)
        halo_top = 1.0 if half == 1 else 0.0
        halo_bot = 1.0 if half == 0 else 0.0
        wpack = base.copy()
        wpack[:, 12] = halo_top * b_v
        wpack[:, 13] = halo_top
        wpack[:, 14] = halo_bot
        in_maps.append({
            "x": np.ascontiguousarray(xr.reshape(C, N).astype(bfloat16)),
            "wpack": wpack, "wpackh": wpackh,
        })
    return in_maps


def kernel(**inputs):
    from concourse.bass_utils import run_bass_kernel_spmd

    if "nc" not in _CACHE:
        _CACHE["nc"] = _build_bass()
    nc = _CACHE["nc"]

    in_maps = _prep_inputs(**inputs)
    res = run_bass_kernel_spmd(nc, in_maps, core_ids=list(range(8)))
    y = np.empty((4, C, HW, HW), dtype=np.float32)
    for core in range(8):
        b, half = core // 2, core % 2
        y0 = 24 * half
        y[b][:, y0:y0 + 24, :] = res.results[core]["out"].reshape(C, 24, HW)
    return y
